# revision 7
# baseline (speedup 1.0000x reference)
"""MoE gate (254 real experts + 254 null copies, top-10) on 8 TRN2 NeuronCores.

Strategy (data-parallel over tokens, per sharding hint):
  - B*T = 16384 tokens sharded 8 ways -> 2048 tokens/core; gate weight replicated.
  - Host pre-transposes x into [d, t]-major tiles so the PE contraction
    (over d) streams from contiguous DMA loads.
  - Per 128-token tile on device:
      PE:  logits[t, e] = sum_d x[t, d] * w[e, d] + bias  (PSUM accumulate)
      ACT: probs = exp(logits)  (unnormalized; also row-sum s_r via accum_out)
      DVE: top-8 via max8 + match_replace, next-8 via second max8, indices
           via max_index (ties resolve to ascending index, matching jax).
      PE:  P_real partial  = recip(s_r)^T @ probs  -> [1, 254] PSUM accumulate
           counts partial  = ones^T @ (probs >= thr) -> [1, 254] PSUM accumulate
      ACT: lse = ln(s_r + 254 * exp(null_logit))
  - Null-expert handling, weight normalization and output packing done once
    per core on [128, 16, 10] strips.
  - Scalar aux-loss terms are reduced on host from per-core partials
    (the "all-reduce" of counts / P_real sums / lse^2 / null counts).
"""

import numpy as np

E = 254
K = 10
D = 2048
B, T = 4, 4096
N_CORES = 8
TPC = (B * T) // N_CORES  # tokens per core = 2048
P = 128                   # partition size (tokens per tile)
NT = TPC // P             # 16 token tiles per core
NK = D // P               # 16 contraction chunks
RHO = 0.5

_NC = None


def _body_once(nc, tc, pools, consts, dram, mybir):
    dt = mybir.dt
    Alu = mybir.AluOpType
    Act = mybir.ActivationFunctionType
    Axis = mybir.AxisListType

    (xpool, ppool, zpool, ipool, spool, stpool, s2pool, pslg, psacc) = pools
    (wt_sb, bias_sb, ones_col, ones_row, iota10, pn_bc, zn_bc) = consts
    (xh_ap, idx_o, w_o, n_o, part_o) = dram

    # ---- strips filled across the token-tile loop ----
    vals16 = stpool.tile([P, NT, 16], dt.float32, tag="vals16")
    idx16 = stpool.tile([P, NT, 16], dt.uint32, tag="idx16")
    lse_st = stpool.tile([P, NT], dt.float32, tag="lse_st")

    prow_ps = psacc.tile([1, E], dt.float32, tag="prow")
    cnt_ps = psacc.tile([1, E], dt.float32, tag="cnt")

    # ---- main loop over 16 token tiles ----
    for i in range(NT):
        x_sb = xpool.tile([P, NK, P], dt.float32, tag="x_sb")
        nc.sync.dma_start(x_sb[:], xh_ap[i])

        lg_ps = pslg.tile([P, E], dt.float32, tag="lg")
        for kt in range(NK):
            nc.tensor.matmul(lg_ps[:], lhsT=x_sb[:, kt, :],
                             rhs=wt_sb[:, kt, :],
                             start=(kt == 0), stop=False)
        nc.tensor.matmul(lg_ps[:], lhsT=ones_row[:], rhs=bias_sb[:],
                         start=False, stop=True)

        probs = ppool.tile([P, E], dt.float32, tag="probs")
        sr = spool.tile([P, 1], dt.float32, tag="sr")
        nc.scalar.activation(probs[:], lg_ps[:], Act.Exp, accum_out=sr[:])
        rsr = spool.tile([P, 1], dt.float32, tag="rsr")
        nc.vector.reciprocal(rsr[:], sr[:])

        r1 = vals16[:, i, 0:8]
        r2 = vals16[:, i, 8:16]
        nc.vector.max(out=r1, in_=probs[:])
        pz = zpool.tile([P, E], dt.float32, tag="pz")
        nc.vector.match_replace(out=pz[:], in_to_replace=r1,
                                in_values=probs[:], imm_value=0.0)
        nc.vector.max(out=r2, in_=pz[:])
        nc.vector.max_index(out=idx16[:, i, 0:8], in_max=r1,
                            in_values=probs[:])
        nc.vector.max_index(out=idx16[:, i, 8:16], in_max=r2,
                            in_values=pz[:])

        thr = spool.tile([P, 1], dt.float32, tag="thr")
        nc.vector.tensor_max(thr[:], vals16[:, i, 9:10], pn_bc[:])
        ind = ipool.tile([P, E], dt.float32, tag="ind")
        nc.vector.tensor_scalar(ind[:], probs[:], thr[:], None,
                                op0=Alu.is_ge)

        nc.tensor.matmul(prow_ps[:], lhsT=rsr[:], rhs=probs[:],
                         start=(i == 0), stop=(i == NT - 1),
                         skip_group_check=True)
        nc.tensor.matmul(cnt_ps[:], lhsT=ones_col[:], rhs=ind[:],
                         start=(i == 0), stop=(i == NT - 1),
                         skip_group_check=True)

        nc.scalar.activation(lse_st[:, i:i + 1], sr[:], Act.Ln,
                             bias=zn_bc[:])

    # ---- stage 2: per-core strip postprocess ----
    v10 = vals16[:, :, 0:10]
    i10 = idx16[:, :, 0:10]

    mask = s2pool.tile([P, NT, K], dt.float32, tag="mask")
    nc.vector.tensor_scalar(mask[:], v10, pn_bc[:], None, op0=Alu.is_ge)
    m_st = s2pool.tile([P, NT], dt.float32, tag="m_st")
    nc.vector.tensor_reduce(m_st[:], mask[:], axis=Axis.X, op=Alu.add)

    wsel = s2pool.tile([P, NT, K], dt.float32, tag="wsel")
    nc.vector.tensor_mul(wsel[:], v10, mask[:])
    ssel = s2pool.tile([P, NT], dt.float32, tag="ssel")
    nc.vector.tensor_reduce(ssel[:], wsel[:], axis=Axis.X, op=Alu.add)
    nc.vector.tensor_scalar_max(ssel[:], ssel[:], 1e-30)
    rsel = s2pool.tile([P, NT], dt.float32, tag="rsel")
    nc.vector.reciprocal(rsel[:], ssel[:])
    wout = s2pool.tile([P, NT, K], dt.float32, tag="wout")
    nc.vector.tensor_mul(wout[:], wsel[:], rsel[:].to_broadcast([P, NT, K]))

    # null-slot indices: 254 + (slot_j - m)
    nm = s2pool.tile([P, NT], dt.float32, tag="nm")
    nc.vector.tensor_scalar(nm[:], m_st[:], -1.0, float(E),
                            op0=Alu.mult, op1=Alu.add)
    nidx = s2pool.tile([P, NT, K], dt.float32, tag="nidx")
    nc.vector.tensor_tensor(
        nidx[:],
        iota10[:].rearrange("p (n k) -> p n k", n=1).to_broadcast([P, NT, K]),
        nm[:].to_broadcast([P, NT, K]),
        op=Alu.add)

    idxf = s2pool.tile([P, NT, K], dt.float32, tag="idxf")
    nc.vector.tensor_copy(idxf[:], i10)
    mask_u8 = s2pool.tile([P, NT, K], dt.uint8, tag="mask_u8")
    nc.vector.tensor_copy(mask_u8[:], mask[:])
    idxo = s2pool.tile([P, NT, K], dt.float32, tag="idxo")
    nc.vector.select(idxo[:], mask_u8[:], idxf[:], nidx[:])
    idxo_i = s2pool.tile([P, NT, K], dt.int32, tag="idxo_i")
    nc.vector.tensor_copy(idxo_i[:], idxo[:])
    inul = s2pool.tile([P, NT, K], dt.uint8, tag="inul")
    nc.vector.tensor_scalar(inul[:], mask[:], -1.0, 1.0,
                            op0=Alu.mult, op1=Alu.add)

    sq = s2pool.tile([P, NT], dt.float32, tag="sq")
    nc.vector.tensor_mul(sq[:], lse_st[:], lse_st[:])

    fin_ps = psacc.tile([1, 2 * NT], dt.float32, tag="fin")
    nc.tensor.matmul(fin_ps[:, 0:NT], lhsT=ones_col[:], rhs=sq[:],
                     start=True, stop=True)
    nc.tensor.matmul(fin_ps[:, NT:2 * NT], lhsT=ones_col[:], rhs=m_st[:],
                     start=True, stop=True)

    part_sb = s2pool.tile([1, 544], dt.float32, tag="part")
    nc.vector.tensor_copy(part_sb[:, 0:E], prow_ps[:])
    nc.vector.tensor_copy(part_sb[:, E:2 * E], cnt_ps[:])
    nc.vector.tensor_copy(part_sb[:, 508:508 + NT], fin_ps[:, 0:NT])
    nc.vector.tensor_copy(part_sb[:, 524:524 + NT], fin_ps[:, NT:2 * NT])
    nc.vector.memset(part_sb[:, 540:544], 0.0)

    nc.sync.dma_start(idx_o.ap(), idxo_i[:])
    nc.sync.dma_start(w_o.ap(), wout[:])
    nc.sync.dma_start(n_o.ap(), inul[:])
    nc.sync.dma_start(part_o.ap(), part_sb[:])


def _build_nc(repeat=1):
    import concourse.bacc as bacc
    import concourse.tile as tile
    import concourse.mybir as mybir

    dt = mybir.dt
    Act = mybir.ActivationFunctionType

    nc = bacc.Bacc("TRN2", target_bir_lowering=False, debug=False,
                   num_devices=N_CORES)

    # Per-core DRAM tensors (host pre-arranged layouts; see kernel()).
    xh = nc.dram_tensor("xh", [NT, P, NK, P], dt.float32, kind="ExternalInput")
    wh = nc.dram_tensor("wh", [P, NK, E], dt.float32, kind="ExternalInput")
    bias = nc.dram_tensor("bias", [1, E], dt.float32, kind="ExternalInput")
    nl = nc.dram_tensor("nl", [1, 1], dt.float32, kind="ExternalInput")

    idx_o = nc.dram_tensor("idx_o", [P, NT, K], dt.int32, kind="ExternalOutput")
    w_o = nc.dram_tensor("w_o", [P, NT, K], dt.float32, kind="ExternalOutput")
    n_o = nc.dram_tensor("n_o", [P, NT, K], dt.uint8, kind="ExternalOutput")
    part_o = nc.dram_tensor("part_o", [1, 544], dt.float32,
                            kind="ExternalOutput")

    with tile.TileContext(nc) as tc:
        with (
            tc.tile_pool(name="consts", bufs=1) as cpool,
            tc.tile_pool(name="xin", bufs=3) as xpool,
            tc.tile_pool(name="probs", bufs=3) as ppool,
            tc.tile_pool(name="pz", bufs=2) as zpool,
            tc.tile_pool(name="ind", bufs=2) as ipool,
            tc.tile_pool(name="small", bufs=4) as spool,
            tc.tile_pool(name="strips", bufs=1) as stpool,
            tc.tile_pool(name="stage2", bufs=1) as s2pool,
            tc.tile_pool(name="psum_lg", bufs=2, space="PSUM") as pslg,
            tc.tile_pool(name="psum_acc", bufs=1, space="PSUM") as psacc,
        ):
            # ---- constants ----
            wt_sb = cpool.tile([P, NK, E], dt.float32)
            nc.sync.dma_start(wt_sb[:], wh.ap())
            bias_sb = cpool.tile([1, E], dt.float32)
            nc.sync.dma_start(bias_sb[:], bias.ap())
            nl_sb = cpool.tile([1, 1], dt.float32)
            nc.sync.dma_start(nl_sb[:], nl.ap())

            ones_col = cpool.tile([P, 1], dt.float32)
            nc.vector.memset(ones_col[:], 1.0)
            ones_row = cpool.tile([1, P], dt.float32)
            nc.vector.memset(ones_row[:], 1.0)
            iota10 = cpool.tile([P, K], dt.float32)
            nc.gpsimd.iota(iota10[:], pattern=[[1, K]], base=0,
                           channel_multiplier=0,
                           allow_small_or_imprecise_dtypes=True)

            # broadcast null_logit to all partitions via rank-1 matmul
            pn_ps = psacc.tile([P, 1], dt.float32, tag="pn_ps")
            nc.tensor.matmul(pn_ps[:], lhsT=ones_row[:], rhs=nl_sb[:],
                             start=True, stop=True)
            pn_bc = cpool.tile([P, 1], dt.float32)  # exp(null_logit)
            nc.scalar.activation(pn_bc[:], pn_ps[:], Act.Exp)
            zn_bc = cpool.tile([P, 1], dt.float32)  # N_NULL * exp(null_logit)
            nc.vector.tensor_scalar_mul(zn_bc[:], pn_bc[:], float(E))

            pools = (xpool, ppool, zpool, ipool, spool, stpool, s2pool,
                     pslg, psacc)
            consts = (wt_sb, bias_sb, ones_col, ones_row, iota10, pn_bc, zn_bc)
            dram = (xh.ap(), idx_o, w_o, n_o, part_o)
            for _rep in range(repeat):
                _body_once(nc, tc, pools, consts, dram, mybir)

    nc.compile()
    return nc


def get_nc():
    global _NC
    if _NC is None:
        _NC = _build_nc()
    return _NC


def make_in_maps(x, gate_w, logit_bias, null_logit):
    """Host-side shard + relayout. Returns per-core input dicts."""
    x = np.ascontiguousarray(np.asarray(x, dtype=np.float32)).reshape(B * T, D)
    gate_w = np.asarray(gate_w, dtype=np.float32)
    logit_bias = np.asarray(logit_bias, dtype=np.float32)
    null_logit = np.asarray(null_logit, dtype=np.float32)

    # wh[p, kt, e] = gate_w[e, kt*P + p]
    wh = np.ascontiguousarray(gate_w.T.reshape(NK, P, E).transpose(1, 0, 2))
    bias2 = np.ascontiguousarray(logit_bias.reshape(1, E))
    nl2 = np.ascontiguousarray(null_logit.reshape(1, 1))

    in_maps = []
    for c in range(N_CORES):
        xc = x[c * TPC:(c + 1) * TPC]  # [2048, 2048]
        # xh[i, p, kt, t] = xc[i*P + t, kt*P + p]
        xhc = np.ascontiguousarray(
            xc.reshape(NT, P, NK, P).transpose(0, 3, 2, 1))
        in_maps.append({"xh": xhc, "wh": wh, "bias": bias2, "nl": nl2})
    return in_maps


def postprocess(results):
    """Combine per-core outputs into full outputs (host 'all-reduce')."""
    idx_parts, w_parts, n_parts = [], [], []
    prow = np.zeros(E, dtype=np.float64)
    counts = np.zeros(E, dtype=np.float64)
    lse2_sum = 0.0
    m_sum = 0.0
    for c in range(N_CORES):
        r = results[c]
        idx_parts.append(r["idx_o"].transpose(1, 0, 2).reshape(TPC, K))
        w_parts.append(r["w_o"].transpose(1, 0, 2).reshape(TPC, K))
        n_parts.append(r["n_o"].transpose(1, 0, 2).reshape(TPC, K))
        p = r["part_o"].reshape(544)
        prow += p[0:E]
        counts += p[E:2 * E]
        lse2_sum += p[508:524].sum()
        m_sum += p[524:540].sum()

    topk_idx = np.concatenate(idx_parts).reshape(B, T, K).astype(np.int32)
    topk_w = np.concatenate(w_parts).reshape(B, T, K).astype(np.float32)
    is_null = np.concatenate(n_parts).reshape(B, T, K).astype(bool)

    n_tok = B * T
    P_real = (prow / n_tok).astype(np.float32)
    counts_f = counts.astype(np.float32)
    f_real = counts_f / np.maximum(counts_f.sum(), np.float32(1e-6))
    L_bal = np.float32(E) * np.float32((f_real * P_real).sum())
    null_total = n_tok * K - m_sum
    null_rate = np.float32(null_total / (n_tok * K))
    L_null = (null_rate - np.float32(RHO)) ** 2
    L_z = np.float32(lse2_sum / n_tok)
    aux = (np.float32(0.02) * L_bal + np.float32(0.001) * L_z
           + np.float32(0.01) * L_null)
    return topk_idx, topk_w, is_null, np.float32(aux)


def kernel(x, gate_w, logit_bias, null_logit):
    from concourse.bass_utils import run_bass_kernel_spmd
    nc = get_nc()
    in_maps = make_in_maps(x, gate_w, logit_bias, null_logit)
    res = run_bass_kernel_spmd(nc, in_maps, core_ids=list(range(N_CORES)))
    return postprocess(res.results)
